# revision 5
# baseline (speedup 1.0000x reference)
"""MultiHeadSINDyAttention TRN2 kernel.

Reference computation (N=4, L=2048, E=512, H=8, h=64, FORECAST=8, DT=1):
    qkv = query @ Wqkv + bqkv ; q,k,v split into 8 heads of 64
    attn = causal-softmax(q k^T / 8) v                    per (batch, head)
    A_h = Xi_h - Xi_h^T ; x_j = attn (I+A_h)^j, j=1..8    (SINDy rollout)
    out[b, j] = concat_h(x_{j,h}) @ Wo + bo               [4, 8, 2048, 512]

Key algebraic fold: the rollout + output projection collapse into
    out[b, j] = sum_h attn_{b,h} @ Wt[j,h] + bo,  Wt[j,h] = (I+A_h)^j Wo_h
so the 8 sequential SINDy steps become 8 precomputed [512, 512] weights
(tiny host-side compute) and the device kernel is three dense matmul
stages + one causal-softmax attention stage.

Sharding: 8 cores = (batch b in 0..3) x (forecast half g in 0..1).
Each core computes attention only for its own 4 heads; the resulting
attnT is exchanged within core pairs (2b, 2b+1) via AllGather so each
core can run the output projection (which contracts over all 8 heads)
for its 4 forecast steps. Outputs are disjoint slices of the full
[4, 8, 2048, 512] result - the gather is pure concatenation.

Differences vs the first working version (316us cost-model time):
  - The pairwise attnT AllGather is split into FOUR per-512-column
    chunks (bf16), each issued as soon as that sequence quarter's
    attention is normalized. Stage D consumes chunks in landing order,
    so the collectives (15us fixed + 512KB/40GBps each in the model)
    hide almost entirely under attention + out-proj compute instead of
    serializing ~135us of PE idle in the middle of the kernel.
  - All matmul *moving* operands are bf16 (qT for the projections,
    qrow for S, es for P@v, wts for out-proj): the cost model charges
    fp32r moving operands 4 cycles/row whenever the output free size
    is < 256, bf16 always 1 cycle/row. Stationaries stay f32r where
    useful; PSUM accumulation is f32 throughout.
  - attnT / gathered attg / Wt / the output tensor are bf16: halves
    the collective payload, the Wt load, and the 16MB output
    writeback (host upconverts to f32; tolerance is 2e-2).
  - Tiles that feed the chunked pipeline (qrow/krow per 512-block,
    v1 per 128-seq-tile, attnT per (pair, quarter)) are separate
    small tiles so whole-tile dependency tracking never serializes a
    chunk on later compute.
  - Engine split: exp on ACT; A/B/normalize work on DVE; stage-D
    PSUM->SBUF staging alternates DVE/ACT; input loads + output
    stores on the SP DMA queue; ccin/collective/attg on the Pool
    queue only (an in-order queue entry that waits on a collective
    must not block unrelated copies).
On-device layout (per core): everything is computed "transposed"
(channels on partitions, sequence on the free axis) so that softmax's
P @ v runs without any transposes:
    qkT[c, s]  = Wqkv^T query^T        (lhsT = Wqkv slices, rhs = query^T)
    S_T[k, q]  = k_h q_h^T             (lhsT = kT_h, rhs = qT_h, K=64)
    E = exp(S_T / 8)                   (ACT, staircase-causal subranges)
    attnT[d|1, q] = [v_h | 1]^T E      (K=128 k-tiles; row 64 = rowsum D)
    attnT_h /= D                       (recip + PE ones-outer broadcast)
    out[q, e]  = attnT^T Wt[j]         (lhsT = attnT, K=512 channels)
Causality at 128-granularity: for the k-tile crossing the diagonal at
offset j*128, only q-columns >= j*128 are computed and a [128,128]
triangle mask (bf16 tensor_mul on the exp'd tile) handles the diagonal.
"""

import os
import sys

for _p in ("/opt/trn_rl_repo", "/root/.axon_site/_ro/trn_rl_repo"):
    if os.path.isdir(_p) and _p not in sys.path:
        sys.path.insert(0, _p)

import numpy as np
import ml_dtypes

import concourse.bass as bass
import concourse.mybir as mybir
from concourse.tile import TileContext
from concourse.bass_utils import run_bass_kernel_spmd

F32 = mybir.dt.float32
F32R = mybir.dt.float32r
BF16 = mybir.dt.bfloat16
AF = mybir.ActivationFunctionType

N_B, L, E, H, EH, FC = 4, 2048, 512, 8, 64, 8
NCORES = 8
KT = E // 128        # 4 k-tiles of 128 over the embedding dim
MT = L // 128        # 16 tiles of 128 over the sequence
QB = L // 512        # 4 query blocks of 512 == collective chunks
HL = H // 2          # local heads per core
EL = HL * EH         # local channel width (q, k or v)
SCALE = 1.0 / np.sqrt(EH)
GROUPS = [[0, 1], [2, 3], [4, 5], [6, 7]]


def legalize_waits(nc):
    """This toolchain's walrus accepts only ONE sync wait per instruction.
    Split extras onto preceding same-engine NoOps (one wait each)."""
    ctr = 0
    for fn in nc.m.functions:
        for blk in fn.blocks:
            out = []
            changed = False
            for inst in blk.instructions:
                si = inst.sync_info
                if si is not None and len(si.on_wait) > 1:
                    for w in si.on_wait[:-1]:
                        out.append(
                            mybir.InstNoOp(
                                name=f"I-xwait-{ctr}",
                                engine=inst.engine,
                                sync_info=mybir.SyncInfo(
                                    on_wait=[w], on_update=[]
                                ),
                            )
                        )
                        ctr += 1
                    inst.sync_info = mybir.SyncInfo(
                        on_wait=[si.on_wait[-1]], on_update=list(si.on_update)
                    )
                    changed = True
                out.append(inst)
            if changed:
                blk.instructions = out
    return ctr


def build_program(with_bias: bool, ebufs: int = 3, obufs: int = 6):
    nc = bass.Bass(target_bir_lowering=False)

    qT = nc.dram_tensor("qT", [E, L], BF16, kind="ExternalInput")
    wqk = nc.dram_tensor("wqk", [E, 2 * EL], BF16, kind="ExternalInput")
    wv = nc.dram_tensor("wv", [E, EL], BF16, kind="ExternalInput")
    wt = nc.dram_tensor("wt", [FC // 2, E, E], BF16, kind="ExternalInput")
    bqk = nc.dram_tensor("bqk", [1, 2 * EL], F32R, kind="ExternalInput")
    bv = nc.dram_tensor("bv", [1, EL], F32R, kind="ExternalInput")
    bo = nc.dram_tensor("bo", [1, E], F32R, kind="ExternalInput")
    onesr = nc.dram_tensor("onesr", [1, 512], F32R, kind="ExternalInput")
    onesf = nc.dram_tensor("onesf", [1, 64], F32R, kind="ExternalInput")
    trid = nc.dram_tensor("trid", [128, 256], BF16, kind="ExternalInput")
    out_d = nc.dram_tensor("out", [FC // 2, L, E], BF16, kind="ExternalOutput")

    with TileContext(nc) as tc:
        with (
            tc.tile_pool(name="const", bufs=1) as cpool,
            tc.tile_pool(name="big", bufs=1) as big,
            tc.tile_pool(name="es", bufs=ebufs) as esp,
            tc.tile_pool(name="nrm", bufs=4) as nrm,
            tc.tile_pool(name="ost", bufs=obufs) as ostp,
            tc.tile_pool(name="psmm", bufs=2, space="PSUM") as psmm,
            tc.tile_pool(name="pss", bufs=2, space="PSUM") as pss,
            tc.tile_pool(name="psa", bufs=2, space="PSUM") as psa,
            tc.tile_pool(name="dram", bufs=1, space="DRAM") as dramp,
        ):
            # ---- persistent loads (SP queue, startup-latency ordered) --
            qTs, wqs, wks = [], [], []
            for kt in range(KT):
                t = big.tile([128, L], BF16, tag=f"qt{kt}", name=f"qt{kt}")
                nc.sync.dma_start(out=t[:, :], in_=qT[kt * 128:(kt + 1) * 128, :])
                qTs.append(t)
                if kt == 0:
                    # weights for head-pair 0 right behind the first qT
                    # tile so stage A can start ~5us in
                    for which, lst, m in (("q", wqs, 0), ("k", wks, 2)):
                        wtile = big.tile([128, KT, 128], BF16,
                                         tag=f"w{which}0", name=f"w{which}0")
                        nc.sync.dma_start(
                            out=wtile[:, :, :],
                            in_=wqk[:, m * 128:(m + 1) * 128].rearrange(
                                "(kt p) m -> p kt m", p=128),
                        )
                        lst.append(wtile)
            wvs = big.tile([128, KT, EL], BF16, tag="wvs")
            nc.sync.dma_start(
                out=wvs[:, :, :],
                in_=wv.rearrange("(kt p) n -> p kt n", p=128),
            )
            tri = big.tile([128, 256], BF16, tag="tri")
            nc.sync.dma_start(out=tri[:, :], in_=trid[:, :])
            ones_s = cpool.tile([1, 512], F32R, tag="ones")
            nc.sync.dma_start(out=ones_s[0:1, :], in_=onesr[:, :])
            onesf_s = cpool.tile([1, 64], F32R, tag="onesf")
            nc.sync.dma_start(out=onesf_s[0:1, :], in_=onesf[:, :])
            if with_bias:
                bqk_s = cpool.tile([1, 2 * EL], F32R, tag="bqk")
                nc.sync.dma_start(out=bqk_s[0:1, :], in_=bqk[:, :])
                bv_s = cpool.tile([1, EL], F32R, tag="bv")
                nc.sync.dma_start(out=bv_s[0:1, :], in_=bv[:, :])
                bo_s = cpool.tile([1, E], F32R, tag="bo")
                nc.sync.dma_start(out=bo_s[0:1, :], in_=bo[:, :])
            for which, lst, m in (("q", wqs, 1), ("k", wks, 3)):
                wtile = big.tile([128, KT, 128], BF16,
                                 tag=f"w{which}1", name=f"w{which}1")
                nc.sync.dma_start(
                    out=wtile[:, :, :],
                    in_=wqk[:, m * 128:(m + 1) * 128].rearrange(
                        "(kt p) m -> p kt m", p=128),
                )
                lst.append(wtile)
            wts = []
            for n in range(FC // 2):
                t = big.tile([128, KT, E], BF16, tag=f"wt{n}", name=f"wt{n}")
                nc.sync.dma_start(
                    out=t[:, :, :],
                    in_=wt[n].rearrange("(ct p) o -> p ct o", p=128),
                )
                wts.append(t)

            # ---- per-chunk tiles ---------------------------------------
            # qrow/krow: [pair][qb] -> [128, 512]; v1: [mt] -> [128, HL, 65]
            # attnT: [pair][qb] -> [128, 512] bf16 (rows = 2 heads x 64)
            qrow = [[big.tile([128, 512], BF16, tag=f"qr{p}{b}",
                              name=f"qr{p}{b}") for b in range(QB)]
                    for p in range(2)]
            krow = [[big.tile([128, 512], BF16, tag=f"kr{p}{b}",
                              name=f"kr{p}{b}") for b in range(QB)]
                    for p in range(2)]
            v1 = [big.tile([128, HL, EH + 1], BF16, tag=f"v1{mt}",
                           name=f"v1{mt}") for mt in range(MT)]
            for mt in range(MT):
                # ones column for the rowsum D; constant, written once
                nc.gpsimd.memset(v1[mt][:, :, EH:EH + 1], 1.0)
            attnT = [[big.tile([128, 512], BF16, tag=f"at{p}{b}",
                               name=f"at{p}{b}") for b in range(QB)]
                     for p in range(2)]
            attg, ccin, ccout = [], [], []
            for b in range(QB):
                attg.append(big.tile([128, 4, 512], BF16, tag=f"ag{b}",
                                     name=f"ag{b}"))
                ccin.append(dramp.tile([2, 128, 512], BF16, tag=f"ci{b}",
                                       name=f"ci{b}"))
                ccout.append(dramp.tile([2, 2, 128, 512], BF16, tag=f"co{b}",
                                        name=f"co{b}"))

            # ---- pipelined A (q/k proj), B (v proj), C (attention) -----
            for qb in range(QB):
                c0 = qb * 512
                # A block: project q,k columns [c0, c0+512) for both pairs
                for p in range(2):
                    for which, wlist, dst in (("q", wqs, qrow), ("k", wks, krow)):
                        pa = psmm.tile([128, 512], F32, tag="mm")
                        for kt in range(KT):
                            nc.tensor.matmul(
                                pa[:, :],
                                wlist[p][:, kt, :],
                                qTs[kt][:, c0:c0 + 512],
                                start=(kt == 0),
                                stop=(kt == KT - 1) and not with_bias,
                            )
                        if with_bias:
                            m = p if which == "q" else 2 + p
                            nc.tensor.matmul(
                                pa[:, :],
                                bqk_s[0:1, m * 128:(m + 1) * 128],
                                ones_s[0:1, :],
                                start=False, stop=True,
                            )
                        nc.vector.tensor_copy(dst[p][qb][:, :], pa[:, :])

                # B block: v projection for seq tiles 4qb .. 4qb+3
                for mt in range(4 * qb, 4 * qb + 4):
                    pv = psmm.tile([128, 512], F32, tag="mm")
                    for kt in range(KT):
                        nc.tensor.matmul(
                            pv[:, 0:EL],
                            qTs[kt][:, mt * 128:(mt + 1) * 128],
                            wvs[:, kt, :],
                            start=(kt == 0),
                            stop=(kt == KT - 1) and not with_bias,
                        )
                    if with_bias:
                        nc.tensor.matmul(
                            pv[:, 0:EL], ones_s[0:1, 0:128], bv_s[0:1, :],
                            start=False, stop=True,
                        )
                    nc.vector.tensor_copy(
                        v1[mt][:, :, 0:EH],
                        pv[:, 0:EL].rearrange("p (h d) -> p h d", h=HL),
                    )

                # C block: causal attention for all 4 local heads, q block qb
                for p, hh in ((0, 0), (1, 0), (0, 1), (1, 1)):
                    h = 2 * p + hh
                    off = hh * EH
                    pA = psa.tile([EH + 1, 512], F32, tag="attn")
                    # non-crossing k-tiles, two tiles share one exp
                    for kt0 in range(0, 4 * qb, 2):
                        pS = pss.tile([128, 1024], F32, tag="s")
                        for half in range(2):
                            kt = kt0 + half
                            nc.tensor.matmul(
                                pS[:, half * 512:half * 512 + 512],
                                krow[p][kt // 4][off:off + EH,
                                                 (kt % 4) * 128:(kt % 4) * 128 + 128],
                                qrow[p][qb][off:off + EH, :],
                                start=True, stop=True,
                            )
                        es = esp.tile([128, 1024], BF16, tag="es")
                        nc.scalar.activation(
                            es[:, :], pS[:, :], AF.Exp, scale=float(SCALE),
                        )
                        for half in range(2):
                            nc.tensor.matmul(
                                pA[:, :],
                                v1[kt0 + half][:, h, :],
                                es[:, half * 512:(half + 1) * 512],
                                start=(kt0 + half == 0),
                                stop=False,
                            )
                    # crossing k-tiles: only q-cols >= j*128 exist.
                    # Pack (j0,j1) and (j2,j3) into one psum tile each:
                    # one exp + one strided triangle-mul per pack.
                    for ja, jb in ((0, 1), (2, 3)):
                        wa, wb = 512 - 128 * ja, 512 - 128 * jb
                        pS = pss.tile([128, 1024], F32, tag="s")
                        es = esp.tile([128, 1024], BF16, tag="es")
                        for j, base in ((ja, 0), (jb, wa)):
                            kt = 4 * qb + j
                            w = 512 - 128 * j
                            nc.tensor.matmul(
                                pS[:, base:base + w],
                                krow[p][qb][off:off + EH,
                                            j * 128:j * 128 + 128],
                                qrow[p][qb][off:off + EH, 128 * j:512],
                                start=True, stop=True,
                            )
                        nc.scalar.activation(
                            es[:, 0:wa + wb], pS[:, 0:wa + wb], AF.Exp,
                            scale=float(SCALE),
                        )
                        # both tiles' triangles sit at local cols 0 and wa
                        trv = es[:, 0:2 * wa].rearrange(
                            "p (j w) -> p j w", j=2
                        )[:, :, 0:128]
                        nc.vector.tensor_mul(
                            trv, trv,
                            tri[:, :].rearrange("p (j w) -> p j w", j=2),
                        )
                        for j, base in ((ja, 0), (jb, wa)):
                            kt = 4 * qb + j
                            w = 512 - 128 * j
                            nc.tensor.matmul(
                                pA[:, 128 * j:512],
                                v1[kt][:, h, :],
                                es[:, base:base + w],
                                start=(kt == 0),
                                stop=(j == 3),
                            )
                    # normalize: attnT[p][qb][off] = pA[0:64] / D, D = pA[64]
                    invd = nrm.tile([1, 512], F32R, tag="invd")
                    with nc.allow_low_precision(
                        reason="f32r is 32-bit storage; rounding only "
                        "at matmul consumption"
                    ):
                        nc.vector.reciprocal(invd[0:1, :], pA[EH:EH + 1, :])
                    pB = psmm.tile([EH, 512], F32, tag="mm")
                    nc.tensor.matmul(
                        pB[:, :], onesf_s[0:1, :], invd[0:1, :],
                        start=True, stop=True,
                    )
                    sbb = nrm.tile([EH, 512], F32, tag="sbb")
                    nc.scalar.copy(sbb[:, :], pB[:, :])
                    nc.vector.tensor_mul(
                        attnT[p][qb][off:off + EH, :], pA[0:EH, :], sbb[:, :],
                    )

                # chunk collective: exchange this quarter's attnT in-pair
                for p in range(2):
                    nc.sync.dma_start(
                        out=ccin[qb][p, :, :], in_=attnT[p][qb][:, :]
                    )
                nc.gpsimd.collective_compute(
                    "AllGather",
                    mybir.AluOpType.bypass,
                    replica_groups=GROUPS,
                    ins=[ccin[qb][:, :, :].opt()],
                    outs=[ccout[qb][:, :, :, :].opt()],
                )
                nc.gpsimd.dma_start(
                    out=attg[qb][:, :, :],
                    in_=ccout[qb].rearrange("r h p n -> p (r h) n"),
                )

            # ---- stage D: output projection, chunk-major ---------------
            cp_i = 0
            for qc in range(QB):
                for n in range(FC // 2):
                    for mtl in range(4):
                        mt = qc * 4 + mtl
                        pO = psmm.tile([128, 512], F32, tag="mm")
                        for ct in range(KT):
                            nc.tensor.matmul(
                                pO[:, :],
                                attg[qc][:, ct, mtl * 128:(mtl + 1) * 128],
                                wts[n][:, ct, :],
                                start=(ct == 0),
                                stop=(ct == KT - 1) and not with_bias,
                            )
                        if with_bias:
                            nc.tensor.matmul(
                                pO[:, :], ones_s[0:1, 0:128], bo_s[0:1, :],
                                start=False, stop=True,
                            )
                        ost = ostp.tile([128, 512], BF16, tag="ost")
                        if cp_i % 2 == 0:
                            nc.vector.tensor_copy(ost[:, :], pO[:, :])
                        else:
                            nc.scalar.copy(ost[:, :], pO[:, :])
                        cp_i += 1
                        nc.sync.dma_start(
                            out=out_d[n, mt * 128:(mt + 1) * 128, :],
                            in_=ost[:, :],
                        )

    legalize_waits(nc)
    return nc


_PROGRAMS = {}
BEST_KW = dict(ebufs=3, obufs=6)


def _get_program(with_bias: bool):
    key = with_bias
    if key not in _PROGRAMS:
        _PROGRAMS[key] = build_program(with_bias, **BEST_KW)
    return _PROGRAMS[key]


def _host_inputs(query, Wqkv, bqkv, Wo, bo, Xi):
    """Per-core input maps. Core c = (batch c//2, forecast-half c%2)."""
    query = np.asarray(query, np.float32)
    Wqkv = np.asarray(Wqkv, np.float32)
    bqkv = np.asarray(bqkv, np.float32)
    Wo = np.asarray(Wo, np.float32)
    bo = np.asarray(bo, np.float32)
    Xi = np.asarray(Xi, np.float64)

    # Wt[j, h] = (I + Xi_h - Xi_h^T)^(j+1) @ Wo_h, stacked over h.
    A = Xi - np.swapaxes(Xi, -1, -2)
    B = np.eye(EH, dtype=np.float64)[None] + A          # [H, 64, 64]
    Wt = np.empty((FC, E, E), np.float32)
    Bp = np.broadcast_to(np.eye(EH, dtype=np.float64), (H, EH, EH)).copy()
    Wo64 = Wo.astype(np.float64).reshape(H, EH, E)
    for j in range(FC):
        Bp = Bp @ B
        Wt[j] = (Bp @ Wo64).reshape(E, E).astype(np.float32)

    kk = np.arange(128)[:, None]
    qq = np.arange(128)[None, :]
    tri1 = (qq >= kk).astype(ml_dtypes.bfloat16)
    tri = np.concatenate([tri1, tri1], axis=1)  # [128, 256], two triangles

    onesr = np.ones((1, 512), np.float32)
    onesf = np.ones((1, 64), np.float32)
    bo_r = bo.reshape(1, -1)
    with_bias = bool(np.any(bqkv) or np.any(bo))

    in_maps = []
    for c in range(NCORES):
        b, g = c // 2, c % 2
        # this core owns heads 4g..4g+3: their q, k, v channel slices
        qs, ks, vs = (slice(g * EL, (g + 1) * EL),
                      slice(E + g * EL, E + (g + 1) * EL),
                      slice(2 * E + g * EL, 2 * E + (g + 1) * EL))
        wqk = np.ascontiguousarray(
            np.concatenate([Wqkv[:, qs], Wqkv[:, ks]], axis=1))
        wv_ = np.ascontiguousarray(Wqkv[:, vs])
        bqk_ = np.concatenate([bqkv[qs], bqkv[ks]]).reshape(1, -1)
        bv_ = np.ascontiguousarray(bqkv[vs]).reshape(1, -1)
        in_maps.append({
            "qT": np.ascontiguousarray(query[b].T).astype(ml_dtypes.bfloat16),
            "wqk": wqk.astype(ml_dtypes.bfloat16),
            "wv": wv_.astype(ml_dtypes.bfloat16),
            "wt": np.ascontiguousarray(Wt[4 * g: 4 * g + 4]).astype(
                ml_dtypes.bfloat16),
            "bqk": bqk_,
            "bv": bv_,
            "bo": bo_r,
            "onesr": onesr,
            "onesf": onesf,
            "trid": tri,
        })
    return in_maps, with_bias


def _run(in_maps, with_bias, **kw):
    nc = _get_program(with_bias)
    return run_bass_kernel_spmd(nc, in_maps, list(range(NCORES)), **kw)


def kernel(query, key, value, Wqkv, bqkv, Wo, bo, Xi, _res_out=None, **kw):
    in_maps, with_bias = _host_inputs(query, Wqkv, bqkv, Wo, bo, Xi)
    res = _run(in_maps, with_bias, **kw)
    if _res_out is not None:
        _res_out.append(res)
    full = np.empty((N_B, FC, L, E), np.float32)
    for c in range(NCORES):
        b, g = c // 2, c % 2
        full[b, 4 * g: 4 * g + 4] = res.results[c]["out"].astype(np.float32)
    return full


# revision 7
# speedup vs baseline: 1.0208x; 1.0208x over previous
"""MultiHeadSINDyAttention TRN2 kernel.

Reference computation (N=4, L=2048, E=512, H=8, h=64, FORECAST=8, DT=1):
    qkv = query @ Wqkv + bqkv ; q,k,v split into 8 heads of 64
    attn = causal-softmax(q k^T / 8) v                    per (batch, head)
    A_h = Xi_h - Xi_h^T ; x_j = attn (I+A_h)^j, j=1..8    (SINDy rollout)
    out[b, j] = concat_h(x_{j,h}) @ Wo + bo               [4, 8, 2048, 512]

Key algebraic fold: the rollout + output projection collapse into
    out[b, j] = sum_h attn_{b,h} @ Wt[j,h] + bo,  Wt[j,h] = (I+A_h)^j Wo_h
so the 8 sequential SINDy steps become 8 precomputed [512, 512] weights
(tiny host-side compute) and the device kernel is three dense matmul
stages + one causal-softmax attention stage.

Sharding: 8 cores = (batch b in 0..3) x (forecast half g in 0..1).
Each core computes attention only for its own 4 heads; the resulting
attnT is exchanged within core pairs (2b, 2b+1) via AllGather so each
core can run the output projection (which contracts over all 8 heads)
for its 4 forecast steps. Outputs are disjoint slices of the full
[4, 8, 2048, 512] result - the gather is pure concatenation.

Differences vs the first working version (316us cost-model time):
  - The pairwise attnT AllGather is split into FOUR per-512-column
    chunks (bf16), each issued as soon as that sequence quarter's
    attention is normalized. Stage D consumes chunks in landing order,
    so the collectives (15us fixed + 512KB/40GBps each in the model)
    hide almost entirely under attention + out-proj compute instead of
    serializing ~135us of PE idle in the middle of the kernel.
  - All matmul *moving* operands are bf16 (qT for the projections,
    qrow for S, es for P@v, wts for out-proj): the cost model charges
    fp32r moving operands 4 cycles/row whenever the output free size
    is < 256, bf16 always 1 cycle/row. Stationaries stay f32r where
    useful; PSUM accumulation is f32 throughout.
  - attnT / gathered attg / Wt / the output tensor are bf16: halves
    the collective payload, the Wt load, and the 16MB output
    writeback (host upconverts to f32; tolerance is 2e-2).
  - Tiles that feed the chunked pipeline (qrow/krow per 512-block,
    v1 per 128-seq-tile, attnT per (pair, quarter)) are separate
    small tiles so whole-tile dependency tracking never serializes a
    chunk on later compute.
  - Engine split: exp on ACT; A/B/normalize work on DVE; stage-D
    PSUM->SBUF staging alternates DVE/ACT; input loads + output
    stores on the SP DMA queue; ccin/collective/attg on the Pool
    queue only (an in-order queue entry that waits on a collective
    must not block unrelated copies).
On-device layout (per core): everything is computed "transposed"
(channels on partitions, sequence on the free axis) so that softmax's
P @ v runs without any transposes:
    qkT[c, s]  = Wqkv^T query^T        (lhsT = Wqkv slices, rhs = query^T)
    S_T[k, q]  = k_h q_h^T             (lhsT = kT_h, rhs = qT_h, K=64)
    E = exp(S_T / 8)                   (ACT, staircase-causal subranges)
    attnT[d|1, q] = [v_h | 1]^T E      (K=128 k-tiles; row 64 = rowsum D)
    attnT_h /= D                       (recip + PE ones-outer broadcast)
    out[q, e]  = attnT^T Wt[j]         (lhsT = attnT, K=512 channels)
Causality at 128-granularity: for the k-tile crossing the diagonal at
offset j*128, only q-columns >= j*128 are computed and a [128,128]
triangle mask (bf16 tensor_mul on the exp'd tile) handles the diagonal.
"""

import os
import sys

for _p in ("/opt/trn_rl_repo", "/root/.axon_site/_ro/trn_rl_repo"):
    if os.path.isdir(_p) and _p not in sys.path:
        sys.path.insert(0, _p)

import numpy as np
import ml_dtypes

import concourse.bass as bass
import concourse.mybir as mybir
from concourse.tile import TileContext
from concourse.bass_utils import run_bass_kernel_spmd

F32 = mybir.dt.float32
F32R = mybir.dt.float32r
BF16 = mybir.dt.bfloat16
AF = mybir.ActivationFunctionType

N_B, L, E, H, EH, FC = 4, 2048, 512, 8, 64, 8
NCORES = 8
KT = E // 128        # 4 k-tiles of 128 over the embedding dim
MT = L // 128        # 16 tiles of 128 over the sequence
QB = L // 512        # 4 query blocks of 512 == collective chunks
HL = H // 2          # local heads per core
EL = HL * EH         # local channel width (q, k or v)
SCALE = 1.0 / np.sqrt(EH)
GROUPS = [[0, 1], [2, 3], [4, 5], [6, 7]]


def legalize_waits(nc):
    """This toolchain's walrus accepts only ONE sync wait per instruction.
    Split extras onto preceding same-engine NoOps (one wait each)."""
    ctr = 0
    for fn in nc.m.functions:
        for blk in fn.blocks:
            out = []
            changed = False
            for inst in blk.instructions:
                si = inst.sync_info
                if si is not None and len(si.on_wait) > 1:
                    for w in si.on_wait[:-1]:
                        out.append(
                            mybir.InstNoOp(
                                name=f"I-xwait-{ctr}",
                                engine=inst.engine,
                                sync_info=mybir.SyncInfo(
                                    on_wait=[w], on_update=[]
                                ),
                            )
                        )
                        ctr += 1
                    inst.sync_info = mybir.SyncInfo(
                        on_wait=[si.on_wait[-1]], on_update=list(si.on_update)
                    )
                    changed = True
                out.append(inst)
            if changed:
                blk.instructions = out
    return ctr


def build_program(with_bias: bool, ebufs: int = 3, obufs: int = 6):
    nc = bass.Bass(target_bir_lowering=False)

    qT = nc.dram_tensor("qT", [E, L], BF16, kind="ExternalInput")
    wqk = nc.dram_tensor("wqk", [E, 2 * EL], BF16, kind="ExternalInput")
    wv = nc.dram_tensor("wv", [E, EL], BF16, kind="ExternalInput")
    wt = nc.dram_tensor("wt", [FC // 2, E, E], BF16, kind="ExternalInput")
    bqk = nc.dram_tensor("bqk", [1, 2 * EL], F32R, kind="ExternalInput")
    bv = nc.dram_tensor("bv", [1, EL], F32R, kind="ExternalInput")
    bo = nc.dram_tensor("bo", [1, E], F32R, kind="ExternalInput")
    onesr = nc.dram_tensor("onesr", [1, 512], F32R, kind="ExternalInput")
    onesf = nc.dram_tensor("onesf", [1, 64], F32R, kind="ExternalInput")
    negm_d = nc.dram_tensor("negm", [128, 128], BF16, kind="ExternalInput")
    iden_d = nc.dram_tensor("iden", [128, 128], BF16, kind="ExternalInput")
    out_d = nc.dram_tensor("out", [FC // 2, L, E], BF16, kind="ExternalOutput")

    with TileContext(nc) as tc:
        with (
            tc.tile_pool(name="const", bufs=1) as cpool,
            tc.tile_pool(name="big", bufs=1) as big,
            tc.tile_pool(name="es", bufs=ebufs) as esp,
            tc.tile_pool(name="nrm", bufs=4) as nrm,
            tc.tile_pool(name="ost", bufs=obufs) as ostp,
            tc.tile_pool(name="psmm", bufs=2, space="PSUM") as psmm,
            tc.tile_pool(name="pss", bufs=2, space="PSUM") as pss,
            tc.tile_pool(name="psa", bufs=2, space="PSUM") as psa,
            tc.tile_pool(name="dram", bufs=1, space="DRAM") as dramp,
        ):
            # ---- persistent loads (SP queue, startup-latency ordered) --
            # qT arrives in per-(kt, qb) 512-column chunks: stage A/B for
            # qb0 only needs the first four, so PE starts ~2.8us in.
            qTs = [[None] * QB for _ in range(KT)]
            wqs, wks = [], []

            def load_qchunk(b):
                for kt in range(KT):
                    t = big.tile([128, 512], BF16, tag=f"qt{kt}_{b}",
                                 name=f"qt{kt}_{b}")
                    nc.sync.dma_start(
                        out=t[:, :],
                        in_=qT[kt * 128:(kt + 1) * 128, b * 512:(b + 1) * 512])
                    qTs[kt][b] = t

            load_qchunk(0)
            for p in range(2):
                for which, lst, m in (("q", wqs, p), ("k", wks, 2 + p)):
                    wtile = big.tile([128, KT, 128], BF16,
                                     tag=f"w{which}{p}", name=f"w{which}{p}")
                    nc.sync.dma_start(
                        out=wtile[:, :, :],
                        in_=wqk[:, m * 128:(m + 1) * 128].rearrange(
                            "(kt p) m -> p kt m", p=128),
                    )
                    lst.append(wtile)
            wvs = big.tile([128, KT, EL], BF16, tag="wvs")
            nc.sync.dma_start(
                out=wvs[:, :, :],
                in_=wv.rearrange("(kt p) n -> p kt n", p=128),
            )
            load_qchunk(1)
            negm = big.tile([128, 128], BF16, tag="negm")
            nc.sync.dma_start(out=negm[:, :], in_=negm_d[:, :])
            iden = big.tile([128, 128], BF16, tag="iden")
            nc.sync.dma_start(out=iden[:, :], in_=iden_d[:, :])
            ones_s = cpool.tile([1, 512], F32R, tag="ones")
            nc.sync.dma_start(out=ones_s[0:1, :], in_=onesr[:, :])
            onesf_s = cpool.tile([1, 64], F32R, tag="onesf")
            nc.sync.dma_start(out=onesf_s[0:1, :], in_=onesf[:, :])
            if with_bias:
                bqk_s = cpool.tile([1, 2 * EL], F32R, tag="bqk")
                nc.sync.dma_start(out=bqk_s[0:1, :], in_=bqk[:, :])
                bv_s = cpool.tile([1, EL], F32R, tag="bv")
                nc.sync.dma_start(out=bv_s[0:1, :], in_=bv[:, :])
                bo_s = cpool.tile([1, E], F32R, tag="bo")
                nc.sync.dma_start(out=bo_s[0:1, :], in_=bo[:, :])
            load_qchunk(2)
            load_qchunk(3)
            wts = []
            for n in range(FC // 2):
                t = big.tile([128, KT, E], BF16, tag=f"wt{n}", name=f"wt{n}")
                nc.sync.dma_start(
                    out=t[:, :, :],
                    in_=wt[n].rearrange("(ct p) o -> p ct o", p=128),
                )
                wts.append(t)

            # ---- per-chunk tiles ---------------------------------------
            # qrow/krow: [pair][qb] -> [128, 512]; v1: [mt] -> [128, HL, 65]
            # attnT: [pair][qb] -> [128, 512] bf16 (rows = 2 heads x 64)
            qrow = [[big.tile([128, 512], BF16, tag=f"qr{p}{b}",
                              name=f"qr{p}{b}") for b in range(QB)]
                    for p in range(2)]
            krow = [[big.tile([128, 512], BF16, tag=f"kr{p}{b}",
                              name=f"kr{p}{b}") for b in range(QB)]
                    for p in range(2)]
            v1 = [big.tile([128, HL, EH + 1], BF16, tag=f"v1{mt}",
                           name=f"v1{mt}") for mt in range(MT)]
            for mt in range(MT):
                # ones column for the rowsum D; constant, written once
                nc.gpsimd.memset(v1[mt][:, :, EH:EH + 1], 1.0)
            attnT = [[big.tile([128, 512], BF16, tag=f"at{p}{b}",
                               name=f"at{p}{b}") for b in range(QB)]
                     for p in range(2)]
            attg, ccin, ccout = [], [], []
            for b in range(QB):
                attg.append(big.tile([128, 4, 512], BF16, tag=f"ag{b}",
                                     name=f"ag{b}"))
                ccin.append(dramp.tile([2, 128, 512], BF16, tag=f"ci{b}",
                                       name=f"ci{b}"))
                ccout.append(dramp.tile([2, 2, 128, 512], BF16, tag=f"co{b}",
                                        name=f"co{b}"))

            # ---- pipelined A (q/k proj), B (v proj), C (attention) -----
            # A/B for chunk b emitted one qb AHEAD of C so PE always has
            # independent work while a chunk's normalize trail drains.
            def ab_block(b):
                c0 = b * 512
                for p in range(2):
                    for which, wlist, dst in (("q", wqs, qrow),
                                              ("k", wks, krow)):
                        pa = psmm.tile([128, 512], F32, tag="mm")
                        for kt in range(KT):
                            nc.tensor.matmul(
                                pa[:, :],
                                wlist[p][:, kt, :],
                                qTs[kt][b][:, :],
                                start=(kt == 0),
                                stop=(kt == KT - 1) and not with_bias,
                            )
                        if with_bias:
                            m = p if which == "q" else 2 + p
                            nc.tensor.matmul(
                                pa[:, :],
                                bqk_s[0:1, m * 128:(m + 1) * 128],
                                ones_s[0:1, :],
                                start=False, stop=True,
                            )
                        nc.vector.tensor_copy(dst[p][b][:, :], pa[:, :])
                for mtl in range(4):
                    mt = 4 * b + mtl
                    pv = psmm.tile([128, 512], F32, tag="mm")
                    for kt in range(KT):
                        nc.tensor.matmul(
                            pv[:, 0:EL],
                            qTs[kt][b][:, mtl * 128:(mtl + 1) * 128],
                            wvs[:, kt, :],
                            start=(kt == 0),
                            stop=(kt == KT - 1) and not with_bias,
                        )
                    if with_bias:
                        nc.tensor.matmul(
                            pv[:, 0:EL], ones_s[0:1, 0:128], bv_s[0:1, :],
                            start=False, stop=True,
                        )
                    nc.vector.tensor_copy(
                        v1[mt][:, :, 0:EH],
                        pv[:, 0:EL].rearrange("p (h d) -> p h d", h=HL),
                    )

            ab_block(0)
            ab_block(1)
            for qb in range(QB):
                # C block: causal attention for all 4 local heads, block qb
                for p, hh in ((0, 0), (1, 0), (0, 1), (1, 1)):
                    h = 2 * p + hh
                    off = hh * EH
                    pA = psa.tile([EH + 1, 512], F32, tag="attn")
                    # non-crossing k-tiles, two tiles share one exp
                    for kt0 in range(0, 4 * qb, 2):
                        pS = pss.tile([128, 1024], F32, tag="s")
                        for half in range(2):
                            kt = kt0 + half
                            nc.tensor.matmul(
                                pS[:, half * 512:half * 512 + 512],
                                krow[p][kt // 4][off:off + EH,
                                                 (kt % 4) * 128:(kt % 4) * 128 + 128],
                                qrow[p][qb][off:off + EH, :],
                                start=True, stop=True,
                            )
                        es = esp.tile([128, 1024], BF16, tag="es")
                        nc.scalar.activation(
                            es[:, :], pS[:, :], AF.Exp, scale=float(SCALE),
                        )
                        for half in range(2):
                            nc.tensor.matmul(
                                pA[:, :],
                                v1[kt0 + half][:, h, :],
                                es[:, half * 512:(half + 1) * 512],
                                start=(kt0 + half == 0),
                                stop=False,
                            )
                    # crossing k-tiles: only q-cols >= j*128 exist. The
                    # diagonal 128-col block of each gets -1e30 added in
                    # PSUM via an identity-matmul (mask lands before exp,
                    # keeping DVE out of the S->exp->Pv path).
                    for ja, jb in ((0, 1), (2, 3)):
                        wa, wb = 512 - 128 * ja, 512 - 128 * jb
                        pS = pss.tile([128, 1024], F32, tag="s")
                        es = esp.tile([128, 1024], BF16, tag="es")
                        for j, base in ((ja, 0), (jb, wa)):
                            kt = 4 * qb + j
                            w = 512 - 128 * j
                            nc.tensor.matmul(
                                pS[:, base:base + w],
                                krow[p][qb][off:off + EH,
                                            j * 128:j * 128 + 128],
                                qrow[p][qb][off:off + EH, 128 * j:512],
                                start=True, stop=False,
                                skip_group_check=True,
                            )
                            nc.tensor.matmul(
                                pS[:, base:base + 128],
                                iden[:, :], negm[:, :],
                                start=False, stop=True,
                                skip_group_check=True,
                            )
                        nc.scalar.activation(
                            es[:, 0:wa + wb], pS[:, 0:wa + wb], AF.Exp,
                            scale=float(SCALE),
                        )
                        for j, base in ((ja, 0), (jb, wa)):
                            kt = 4 * qb + j
                            w = 512 - 128 * j
                            nc.tensor.matmul(
                                pA[:, 128 * j:512],
                                v1[kt][:, h, :],
                                es[:, base:base + w],
                                start=(kt == 0),
                                stop=(j == 3),
                            )
                    # normalize: attnT[p][qb][off] = pA[0:64] / D, D = pA[64]
                    invd = nrm.tile([1, 512], F32R, tag="invd")
                    with nc.allow_low_precision(
                        reason="f32r is 32-bit storage; rounding only "
                        "at matmul consumption"
                    ):
                        nc.vector.reciprocal(invd[0:1, :], pA[EH:EH + 1, :])
                    pB = psmm.tile([EH, 512], F32, tag="mm")
                    nc.tensor.matmul(
                        pB[:, :], onesf_s[0:1, :], invd[0:1, :],
                        start=True, stop=True,
                    )
                    sbb = nrm.tile([EH, 512], F32, tag="sbb")
                    nc.vector.tensor_copy(sbb[:, :], pB[:, :])
                    nc.vector.tensor_mul(
                        attnT[p][qb][off:off + EH, :], pA[0:EH, :], sbb[:, :],
                    )

                # chunk collective: exchange this quarter's attnT in-pair
                for p in range(2):
                    nc.sync.dma_start(
                        out=ccin[qb][p, :, :], in_=attnT[p][qb][:, :]
                    )
                nc.gpsimd.collective_compute(
                    "AllGather",
                    mybir.AluOpType.bypass,
                    replica_groups=GROUPS,
                    ins=[ccin[qb][:, :, :].opt()],
                    outs=[ccout[qb][:, :, :, :].opt()],
                )
                nc.gpsimd.dma_start(
                    out=attg[qb][:, :, :],
                    in_=ccout[qb].rearrange("r h p n -> p (r h) n"),
                )
                if qb + 2 < QB:
                    ab_block(qb + 2)

            # ---- stage D: output projection, chunk-major ---------------
            cp_i = 0
            for qc in range(QB):
                for n in range(FC // 2):
                    for mtl in range(4):
                        mt = qc * 4 + mtl
                        pO = psmm.tile([128, 512], F32, tag="mm")
                        for ct in range(KT):
                            nc.tensor.matmul(
                                pO[:, :],
                                attg[qc][:, ct, mtl * 128:(mtl + 1) * 128],
                                wts[n][:, ct, :],
                                start=(ct == 0),
                                stop=(ct == KT - 1) and not with_bias,
                            )
                        if with_bias:
                            nc.tensor.matmul(
                                pO[:, :], ones_s[0:1, 0:128], bo_s[0:1, :],
                                start=False, stop=True,
                            )
                        ost = ostp.tile([128, 512], BF16, tag="ost")
                        if cp_i % 2 == 0:
                            nc.scalar.copy(ost[:, :], pO[:, :])
                        else:
                            nc.vector.tensor_copy(ost[:, :], pO[:, :])
                        cp_i += 1
                        nc.sync.dma_start(
                            out=out_d[n, mt * 128:(mt + 1) * 128, :],
                            in_=ost[:, :],
                        )

    legalize_waits(nc)
    return nc


_PROGRAMS = {}
BEST_KW = dict(ebufs=3, obufs=6)


def _get_program(with_bias: bool):
    key = with_bias
    if key not in _PROGRAMS:
        _PROGRAMS[key] = build_program(with_bias, **BEST_KW)
    return _PROGRAMS[key]


def _host_inputs(query, Wqkv, bqkv, Wo, bo, Xi):
    """Per-core input maps. Core c = (batch c//2, forecast-half c%2)."""
    query = np.asarray(query, np.float32)
    Wqkv = np.asarray(Wqkv, np.float32)
    bqkv = np.asarray(bqkv, np.float32)
    Wo = np.asarray(Wo, np.float32)
    bo = np.asarray(bo, np.float32)
    Xi = np.asarray(Xi, np.float64)

    # Wt[j, h] = (I + Xi_h - Xi_h^T)^(j+1) @ Wo_h, stacked over h.
    A = Xi - np.swapaxes(Xi, -1, -2)
    B = np.eye(EH, dtype=np.float64)[None] + A          # [H, 64, 64]
    Wt = np.empty((FC, E, E), np.float32)
    Bp = np.broadcast_to(np.eye(EH, dtype=np.float64), (H, EH, EH)).copy()
    Wo64 = Wo.astype(np.float64).reshape(H, EH, E)
    for j in range(FC):
        Bp = Bp @ B
        Wt[j] = (Bp @ Wo64).reshape(E, E).astype(np.float32)

    kk = np.arange(128)[:, None]
    qq = np.arange(128)[None, :]
    negm = np.where(qq >= kk, 0.0, -1e30).astype(ml_dtypes.bfloat16)
    iden = np.eye(128, dtype=ml_dtypes.bfloat16)

    onesr = np.ones((1, 512), np.float32)
    onesf = np.ones((1, 64), np.float32)
    bo_r = bo.reshape(1, -1)
    with_bias = bool(np.any(bqkv) or np.any(bo))

    in_maps = []
    for c in range(NCORES):
        b, g = c // 2, c % 2
        # this core owns heads 4g..4g+3: their q, k, v channel slices
        qs, ks, vs = (slice(g * EL, (g + 1) * EL),
                      slice(E + g * EL, E + (g + 1) * EL),
                      slice(2 * E + g * EL, 2 * E + (g + 1) * EL))
        wqk = np.ascontiguousarray(
            np.concatenate([Wqkv[:, qs], Wqkv[:, ks]], axis=1))
        wv_ = np.ascontiguousarray(Wqkv[:, vs])
        bqk_ = np.concatenate([bqkv[qs], bqkv[ks]]).reshape(1, -1)
        bv_ = np.ascontiguousarray(bqkv[vs]).reshape(1, -1)
        in_maps.append({
            "qT": np.ascontiguousarray(query[b].T).astype(ml_dtypes.bfloat16),
            "wqk": wqk.astype(ml_dtypes.bfloat16),
            "wv": wv_.astype(ml_dtypes.bfloat16),
            "wt": np.ascontiguousarray(Wt[4 * g: 4 * g + 4]).astype(
                ml_dtypes.bfloat16),
            "bqk": bqk_,
            "bv": bv_,
            "bo": bo_r,
            "onesr": onesr,
            "onesf": onesf,
            "negm": negm,
            "iden": iden,
        })
    return in_maps, with_bias


def _run(in_maps, with_bias, **kw):
    nc = _get_program(with_bias)
    return run_bass_kernel_spmd(nc, in_maps, list(range(NCORES)), **kw)


def kernel(query, key, value, Wqkv, bqkv, Wo, bo, Xi, _res_out=None, **kw):
    in_maps, with_bias = _host_inputs(query, Wqkv, bqkv, Wo, bo, Xi)
    res = _run(in_maps, with_bias, **kw)
    if _res_out is not None:
        _res_out.append(res)
    full = np.empty((N_B, FC, L, E), np.float32)
    for c in range(NCORES):
        b, g = c // 2, c % 2
        full[b, 4 * g: 4 * g + 4] = res.results[c]["out"].astype(np.float32)
    return full


# revision 41
# speedup vs baseline: 1.1112x; 1.0886x over previous
"""MultiHeadSINDyAttention TRN2 kernel.

Reference computation (N=4, L=2048, E=512, H=8, h=64, FORECAST=8, DT=1):
    qkv = query @ Wqkv + bqkv ; q,k,v split into 8 heads of 64
    attn = causal-softmax(q k^T / 8) v                    per (batch, head)
    A_h = Xi_h - Xi_h^T ; x_j = attn (I+A_h)^j, j=1..8    (SINDy rollout)
    out[b, j] = concat_h(x_{j,h}) @ Wo + bo               [4, 8, 2048, 512]

Key algebraic fold: the rollout + output projection collapse into
    out[b, j] = sum_h attn_{b,h} @ Wt[j,h] + bo,  Wt[j,h] = (I+A_h)^j Wo_h
so the 8 sequential SINDy steps become 8 precomputed [512, 512] weights
(tiny host-side compute) and the device kernel is three dense matmul
stages + one causal-softmax attention stage.

Sharding: 8 cores = (batch b in 0..3) x (forecast half g in 0..1).
Each core computes attention only for its own 4 heads; the resulting
attnT is exchanged within core pairs (2b, 2b+1) via AllGather so each
core can run the output projection (which contracts over all 8 heads)
for its 4 forecast steps. Outputs are disjoint slices of the full
[4, 8, 2048, 512] result - the gather is pure concatenation.

Differences vs the first working version (316us cost-model time):
  - The pairwise attnT AllGather is split into FOUR per-512-column
    chunks (bf16), each issued as soon as that sequence quarter's
    attention is normalized. Stage D consumes chunks in landing order,
    so the collectives (15us fixed + 512KB/40GBps each in the model)
    hide almost entirely under attention + out-proj compute instead of
    serializing ~135us of PE idle in the middle of the kernel.
  - All matmul *moving* operands are bf16 (qT for the projections,
    qrow for S, es for P@v, wts for out-proj): the cost model charges
    fp32r moving operands 4 cycles/row whenever the output free size
    is < 256, bf16 always 1 cycle/row. Stationaries stay f32r where
    useful; PSUM accumulation is f32 throughout.
  - attnT / gathered attg / Wt / the output tensor are bf16: halves
    the collective payload, the Wt load, and the 16MB output
    writeback (host upconverts to f32; tolerance is 2e-2).
  - Tiles that feed the chunked pipeline (qrow/krow per 512-block,
    v1 per 128-seq-tile, attnT per (pair, quarter)) are separate
    small tiles so whole-tile dependency tracking never serializes a
    chunk on later compute.
  - The causal triangle on each diagonal block is applied by ADDING
    -1e30 into PSUM with an identity-matmul (lhsT=I, rhs=mask) before
    the exp, so the S -> exp -> P@v chain never touches DVE.
  - Engine split: exp on ACT; A/B copies + normalize on DVE; stage-D
    PSUM->SBUF staging alternates ACT/DVE; all loads + ccin + output
    stores on the SP HWDGE ring (ordered by first use - each DMA costs
    ~0.65us of ring dispatch, so order matters at startup); the
    collectives + attg unload on the Pool queue. DMA dispatch is
    async (descriptors wait their semaphores in the ring), so a
    waiting DMA never head-of-line-blocks a queue.
On-device layout (per core): everything is computed "transposed"
(channels on partitions, sequence on the free axis) so that softmax's
P @ v runs without any transposes:
    qkT[c, s]  = Wqkv^T query^T        (lhsT = Wqkv slices, rhs = query^T)
    S_T[k, q]  = k_h q_h^T             (lhsT = kT_h, rhs = qT_h, K=64)
    E = exp(S_T / 8)                   (ACT, staircase-causal subranges)
    attnT[d|1, q] = [v_h | 1]^T E      (K=128 k-tiles; row 64 = rowsum D)
    attnT_h /= D                       (recip + PE ones-outer broadcast)
    out[q, e]  = attnT^T Wt[j]         (lhsT = attnT, K=512 channels)
Causality at 128-granularity: for the k-tile crossing the diagonal at
offset j*128, only q-columns >= j*128 are computed and the [128,128]
diagonal block is masked by the PSUM -1e30 add described above.
Cost-model time: 316.6us (first working version) -> 175.6us.
Stage-D chains alternate PSUM between the psmm pool and the pss pool
(idle once attention ends) for 4 rotating slots, with per-chain
ACT/DVE copy alternation and a split final store to shorten the tail.
"""

import os
import sys

for _p in ("/opt/trn_rl_repo", "/root/.axon_site/_ro/trn_rl_repo"):
    if os.path.isdir(_p) and _p not in sys.path:
        sys.path.insert(0, _p)

import numpy as np
import ml_dtypes

import concourse.bass as bass
import concourse.mybir as mybir
from concourse.tile import TileContext
from concourse.bass_utils import run_bass_kernel_spmd

F32 = mybir.dt.float32
F32R = mybir.dt.float32r
BF16 = mybir.dt.bfloat16
AF = mybir.ActivationFunctionType

N_B, L, E, H, EH, FC = 4, 2048, 512, 8, 64, 8
NCORES = 8
KT = E // 128        # 4 k-tiles of 128 over the embedding dim
MT = L // 128        # 16 tiles of 128 over the sequence
QB = L // 512        # 4 query blocks of 512 == collective chunks
HL = H // 2          # local heads per core
EL = HL * EH         # local channel width (q, k or v)
SCALE = 1.0 / np.sqrt(EH)
GROUPS = [[0, 1], [2, 3], [4, 5], [6, 7]]


def legalize_waits(nc):
    """This toolchain's walrus accepts only ONE sync wait per instruction.
    Split extras onto preceding same-engine NoOps (one wait each)."""
    ctr = 0
    for fn in nc.m.functions:
        for blk in fn.blocks:
            out = []
            changed = False
            for inst in blk.instructions:
                si = inst.sync_info
                if si is not None and len(si.on_wait) > 1:
                    for w in si.on_wait[:-1]:
                        out.append(
                            mybir.InstNoOp(
                                name=f"I-xwait-{ctr}",
                                engine=inst.engine,
                                sync_info=mybir.SyncInfo(
                                    on_wait=[w], on_update=[]
                                ),
                            )
                        )
                        ctr += 1
                    inst.sync_info = mybir.SyncInfo(
                        on_wait=[si.on_wait[-1]], on_update=list(si.on_update)
                    )
                    changed = True
                out.append(inst)
            if changed:
                blk.instructions = out
    return ctr


def build_program(with_bias: bool, ebufs: int = 3, obufs: int = 6,
                  xsplit: int = -1, sbb_eng: str = "vector", nbufs: int = 4,
                  D_ENGS: tuple = ("scalar", "vector"),
                  HEAD_ORDER: tuple = ((0, 0), (1, 0), (0, 1), (1, 1)),
                  AB_EARLY: int = 0, xsplit01: bool = False,
                  AB_PSS: bool = False, IL2: bool = False,
                  DEFER: bool = True, B_FIRST: bool = False,
                  p23_psmm: bool = False):
    nc = bass.Bass(target_bir_lowering=False)

    qT = nc.dram_tensor("qT", [E, L], BF16, kind="ExternalInput")
    wqk = nc.dram_tensor("wqk", [E, 2 * EL], BF16, kind="ExternalInput")
    wv = nc.dram_tensor("wv", [E, EL], BF16, kind="ExternalInput")
    wt = nc.dram_tensor("wt", [FC // 2, E, E], BF16, kind="ExternalInput")
    bqk = nc.dram_tensor("bqk", [1, 2 * EL], F32R, kind="ExternalInput")
    bv = nc.dram_tensor("bv", [1, EL], F32R, kind="ExternalInput")
    bo = nc.dram_tensor("bo", [1, E], F32R, kind="ExternalInput")
    onesr = nc.dram_tensor("onesr", [1, 512], F32R, kind="ExternalInput")
    onesf = nc.dram_tensor("onesf", [1, 64], F32R, kind="ExternalInput")
    negm_d = nc.dram_tensor("negm", [128, 128], BF16, kind="ExternalInput")
    iden_d = nc.dram_tensor("iden", [128, 128], BF16, kind="ExternalInput")
    out_d = nc.dram_tensor("out", [FC // 2, L, E], BF16, kind="ExternalOutput")

    with TileContext(nc) as tc:
        with (
            tc.tile_pool(name="const", bufs=1) as cpool,
            tc.tile_pool(name="big", bufs=1) as big,
            tc.tile_pool(name="es", bufs=ebufs) as esp,
            tc.tile_pool(name="nrm", bufs=nbufs) as nrm,
            tc.tile_pool(name="ost", bufs=obufs) as ostp,
            tc.tile_pool(name="psmm", bufs=2, space="PSUM") as psmm,
            tc.tile_pool(name="pss", bufs=2, space="PSUM") as pss,
            tc.tile_pool(name="psa", bufs=2, space="PSUM") as psa,
            tc.tile_pool(name="dram", bufs=1, space="DRAM") as dramp,
        ):
            # ---- persistent loads (SP ring, ordered by first use) ------
            qTs = [[None] * QB for _ in range(KT)]
            wqs, wks = [], []

            def load_w(p):
                for which, lst, m in (("q", wqs, p), ("k", wks, 2 + p)):
                    wtile = big.tile([128, KT, 128], BF16,
                                     tag=f"w{which}{p}", name=f"w{which}{p}")
                    nc.sync.dma_start(
                        out=wtile[:, :, :],
                        in_=wqk[:, m * 128:(m + 1) * 128].rearrange(
                            "(kt p) m -> p kt m", p=128),
                    )
                    lst.append(wtile)

            def load_qchunk(b):
                for kt in range(KT):
                    t = big.tile([128, 512], BF16, tag=f"qt{kt}_{b}",
                                 name=f"qt{kt}_{b}")
                    nc.sync.dma_start(
                        out=t[:, :],
                        in_=qT[kt * 128:(kt + 1) * 128,
                               b * 512:(b + 1) * 512])
                    qTs[kt][b] = t

            load_w(0)
            load_qchunk(0)
            load_w(1)
            wvs = big.tile([128, KT, EL], BF16, tag="wvs")
            nc.sync.dma_start(
                out=wvs[:, :, :],
                in_=wv.rearrange("(kt p) n -> p kt n", p=128),
            )
            load_qchunk(1)
            negm = big.tile([128, 128], BF16, tag="negm")
            nc.sync.dma_start(out=negm[:, :], in_=negm_d[:, :])
            iden = big.tile([128, 128], BF16, tag="iden")
            nc.sync.dma_start(out=iden[:, :], in_=iden_d[:, :])
            ones_s = cpool.tile([1, 512], F32R, tag="ones")
            nc.sync.dma_start(out=ones_s[0:1, :], in_=onesr[:, :])
            onesf_s = cpool.tile([1, 64], F32R, tag="onesf")
            nc.sync.dma_start(out=onesf_s[0:1, :], in_=onesf[:, :])
            if with_bias:
                bqk_s = cpool.tile([1, 2 * EL], F32R, tag="bqk")
                nc.sync.dma_start(out=bqk_s[0:1, :], in_=bqk[:, :])
                bv_s = cpool.tile([1, EL], F32R, tag="bv")
                nc.sync.dma_start(out=bv_s[0:1, :], in_=bv[:, :])
                bo_s = cpool.tile([1, E], F32R, tag="bo")
                nc.sync.dma_start(out=bo_s[0:1, :], in_=bo[:, :])

            load_qchunk(2)
            load_qchunk(3)
            wts = []
            for n in range(FC // 2):
                t = big.tile([128, KT, E], BF16, tag=f"wt{n}", name=f"wt{n}")
                nc.sync.dma_start(
                    out=t[:, :, :],
                    in_=wt[n].rearrange("(ct p) o -> p ct o", p=128),
                )
                wts.append(t)

            def qch(kt, b):
                return qTs[kt][b][:, :]

            # ---- per-chunk tiles ---------------------------------------
            # qrow/krow: [pair][qb] -> [128, 512]; v1: [mt] -> [128, HL, 65]
            # attnT: [pair][qb] -> [128, 512] bf16 (rows = 2 heads x 64)
            qrow = [[big.tile([128, 512], BF16, tag=f"qr{p}{b}",
                              name=f"qr{p}{b}") for b in range(QB)]
                    for p in range(2)]
            krow = [[big.tile([128, 512], BF16, tag=f"kr{p}{b}",
                              name=f"kr{p}{b}") for b in range(QB)]
                    for p in range(2)]
            # per head: [64 v-dims | 64 ones-columns] so P@v replicates
            # the softmax denominator D onto pA rows 64:128 - the
            # normalize then needs no PE broadcast at all
            v1 = [big.tile([128, HL, 2 * EH], BF16, tag=f"v1{mt}",
                           name=f"v1{mt}") for mt in range(MT)]
            for mt in range(MT):
                nc.gpsimd.memset(v1[mt][:, :, EH:2 * EH], 1.0)
            attnT = [[big.tile([128, 512], BF16, tag=f"at{p}{b}",
                               name=f"at{p}{b}") for b in range(QB)]
                     for p in range(2)]
            attg, ccin, ccout = [], [], []
            for b in range(QB):
                attg.append(big.tile([128, 4, 512], BF16, tag=f"ag{b}",
                                     name=f"ag{b}"))
                ccin.append(dramp.tile([2, 128, 512], BF16, tag=f"ci{b}",
                                       name=f"ci{b}"))
                ccout.append(dramp.tile([2, 2, 128, 512], BF16, tag=f"co{b}",
                                        name=f"co{b}"))

            dch = [0]

            def d_block(qc, n, copy_eng):
                # out-proj for forecast step n over seq tiles of chunk qc.
                # Chains alternate between the psmm pool and the (free
                # during stage D) pss pool: 4 rotating PSUM slots instead
                # of 2, hiding the copy-drain turnaround.
                for mtl in range(4):
                    mt = qc * 4 + mtl
                    if dch[0] % 2 == 0 or qc == 0 and IL2:
                        pO = psmm.tile([128, 512], F32, tag="mm")
                    else:
                        pO = pss.tile([128, 1024], F32, tag="s")
                    dch[0] += 1
                    for ct in range(KT):
                        nc.tensor.matmul(
                            pO[:, 0:512],
                            attg[qc][:, ct, mtl * 128:(mtl + 1) * 128],
                            wts[n][:, ct, :],
                            start=(ct == 0),
                            stop=(ct == KT - 1) and not with_bias,
                        )
                    if with_bias:
                        nc.tensor.matmul(
                            pO[:, 0:512], ones_s[0:1, 0:128], bo_s[0:1, :],
                            start=False, stop=True,
                        )
                    ost = ostp.tile([128, 512], BF16, tag="ost")
                    last = (qc == QB - 1 and n == FC // 2 - 1 and mtl == 3)
                    if last:
                        # split the final tile across ACT and DVE so the
                        # two half-copies drain in parallel and the first
                        # store overlaps the second copy
                        for hf in range(2):
                            sl = slice(hf * 256, hf * 256 + 256)
                            nc.scalar.copy(ost[:, sl], pO[:, sl])
                            nc.sync.dma_start(
                                out=out_d[n, mt * 128:(mt + 1) * 128, sl],
                                in_=ost[:, sl],
                            )
                        continue
                    ce = copy_eng
                    if ce == "alt":
                        ce = "scalar" if dch[0] % 2 else "vector"
                    if ce == "scalar":
                        nc.scalar.copy(ost[:, :], pO[:, 0:512])
                    elif ce == "gpsimd":
                        nc.gpsimd.tensor_copy(ost[:, :], pO[:, 0:512])
                    else:
                        nc.vector.tensor_copy(ost[:, :], pO[:, 0:512])
                    nc.sync.dma_start(
                        out=out_d[n, mt * 128:(mt + 1) * 128, :],
                        in_=ost[:, :],
                    )

            # ---- pipelined A (q/k proj), B (v proj), C (attention) -----
            # A/B for chunk b emitted one qb AHEAD of C so PE always has
            # independent work while a chunk's normalize trail drains.
            def ab_block(b):
                c0 = b * 512
                def a_part():
                    for p in range(2):
                        for which, wlist, dst in (("q", wqs, qrow),
                                                  ("k", wks, krow)):
                            if AB_PSS and (p + (which == "k")) % 2:
                                pa = pss.tile([128, 1024], F32, tag="s")
                            else:
                                pa = psmm.tile([128, 512], F32, tag="mm")
                            for kt in range(KT):
                                nc.tensor.matmul(
                                    pa[:, 0:512],
                                    wlist[p][:, kt, :],
                                    qch(kt, b),
                                    start=(kt == 0),
                                    stop=(kt == KT - 1) and not with_bias,
                                )
                            if with_bias:
                                m = p if which == "q" else 2 + p
                                nc.tensor.matmul(
                                    pa[:, 0:512],
                                    bqk_s[0:1, m * 128:(m + 1) * 128],
                                    ones_s[0:1, :],
                                    start=False, stop=True,
                                )
                            nc.vector.tensor_copy(dst[p][b][:, :], pa[:, 0:512])
                def b_part():
                    for mtl in range(4):
                        mt = 4 * b + mtl
                        pv = psmm.tile([128, 512], F32, tag="mm")
                        for kt in range(KT):
                            nc.tensor.matmul(
                                pv[:, 0:EL],
                                qch(kt, b)[:, mtl * 128:(mtl + 1) * 128],
                                wvs[:, kt, :],
                                start=(kt == 0),
                                stop=(kt == KT - 1) and not with_bias,
                            )
                        if with_bias:
                            nc.tensor.matmul(
                                pv[:, 0:EL], ones_s[0:1, 0:128], bv_s[0:1, :],
                                start=False, stop=True,
                            )
                        nc.vector.tensor_copy(
                            v1[mt][:, :, 0:EH],
                            pv[:, 0:EL].rearrange("p (h d) -> p h d", h=HL),
                        )
                if B_FIRST:
                    b_part(); a_part()
                else:
                    a_part(); b_part()

            ab_block(0)
            ab_block(1)
            if AB_EARLY == 2:
                ab_block(2)
                ab_block(3)
            for qb in range(QB):
                if AB_EARLY == 1 and qb + 2 < QB:
                    ab_block(qb + 2)
                # C block: causal attention for all 4 local heads, block qb
                # Each head's normalize (pB broadcast + mul) is EMITTED
                # inside the NEXT head's crossing section, so the pB
                # matmul fills the PE hole while that head's first
                # crossing exp drains, instead of stalling on its own
                # reciprocal with nothing else to run.
                def emit_norm(pend):
                    pp, off_, pA_, invd_ = pend
                    nc.vector.tensor_mul(
                        attnT[pp][qb][off_:off_ + EH, :], pA_[0:EH, :],
                        invd_[0:EH, :],
                    )

                pending = None
                for hix, (p, hh) in enumerate(HEAD_ORDER):
                    h = 2 * p + hh
                    off = hh * EH
                    pA = psa.tile([128, 512], F32, tag="attn")
                    # non-crossing k-tiles, two tiles share one exp
                    for kt0 in range(0, 4 * qb, 2):
                        pS = pss.tile([128, 1024], F32, tag="s")
                        for half in range(2):
                            kt = kt0 + half
                            nc.tensor.matmul(
                                pS[:, half * 512:half * 512 + 512],
                                krow[p][kt // 4][off:off + EH,
                                                 (kt % 4) * 128:(kt % 4) * 128 + 128],
                                qrow[p][qb][off:off + EH, :],
                                start=True, stop=True,
                            )
                        es = esp.tile([128, 1024], BF16, tag="es")
                        nc.scalar.activation(
                            es[:, :], pS[:, :], AF.Exp, scale=float(SCALE),
                        )
                        for half in range(2):
                            nc.tensor.matmul(
                                pA[:, :],
                                v1[kt0 + half][:, h, :],
                                es[:, half * 512:(half + 1) * 512],
                                start=(kt0 + half == 0),
                                stop=False,
                            )
                    # crossing k-tiles: only q-cols >= j*128 exist. The
                    # diagonal 128-col block of each gets -1e30 added in
                    # PSUM via an identity-matmul (mask lands before exp,
                    # keeping DVE out of the S->exp->Pv path).
                    ess = {}
                    for ja, jb in ((0, 1), (2, 3)):
                        wa, wb = 512 - 128 * ja, 512 - 128 * jb
                        pS = pss.tile([128, 1024], F32, tag="s")
                        es = esp.tile([128, 1024], BF16, tag="es")
                        ess[ja] = es
                        for j, base in ((ja, 0), (jb, wa)):
                            kt = 4 * qb + j
                            w = 512 - 128 * j
                            nc.tensor.matmul(
                                pS[:, base:base + w],
                                krow[p][qb][off:off + EH,
                                            j * 128:j * 128 + 128],
                                qrow[p][qb][off:off + EH, 128 * j:512],
                                start=True, stop=False,
                                skip_group_check=True,
                            )
                            nc.tensor.matmul(
                                pS[:, base:base + 128],
                                iden[:, :], negm[:, :],
                                start=False, stop=True,
                                skip_group_check=True,
                            )
                        nc.scalar.activation(
                            es[:, 0:wa + wb], pS[:, 0:wa + wb], AF.Exp,
                            scale=float(SCALE),
                        )
                        if ja == 0 and pending is not None:
                            emit_norm(pending)
                            pending = None
                    for ja, jb in ((0, 1), (2, 3)):
                        wa = 512 - 128 * ja
                        for j, base in ((ja, 0), (jb, wa)):
                            kt = 4 * qb + j
                            w = 512 - 128 * j
                            nc.tensor.matmul(
                                pA[:, 128 * j:512],
                                v1[kt][:, h, :],
                                ess[ja][:, base:base + w],
                                start=(kt == 0),
                                stop=(j == 3),
                            )
                    # reciprocal of the 64 replicated D rows (DVE cost is
                    # per-column, so 64 rows price the same as 1); the mul
                    # is deferred into the next head's crossing section
                    invd = nrm.tile([EH, 512], F32R, tag="invd")
                    with nc.allow_low_precision(
                        reason="f32r is 32-bit storage; rounding only "
                        "at matmul consumption"
                    ):
                        nc.vector.reciprocal(invd[0:EH, :], pA[EH:2 * EH, :])
                    pending = (p, off, pA, invd)
                emit_norm(pending)

                # chunk collective: exchange this quarter's attnT in-pair
                for p in range(2):
                    nc.sync.dma_start(
                        out=ccin[qb][p, :, :], in_=attnT[p][qb][:, :]
                    )
                nc.gpsimd.collective_compute(
                    "AllGather",
                    mybir.AluOpType.bypass,
                    replica_groups=GROUPS,
                    ins=[ccin[qb][:, :, :].opt()],
                    outs=[ccout[qb][:, :, :, :].opt()],
                )
                nc.gpsimd.dma_start(
                    out=attg[qb][:, :, :],
                    in_=ccout[qb].rearrange("r h p n -> p (r h) n"),
                )
                if AB_EARLY == 0 and qb + 2 < QB:
                    ab_block(qb + 2)

            # ---- stage D: output projection, chunk-major so each
            # gathered quarter is consumed as it lands ------------------
            cp_i = 0
            for qc in range(1 if IL2 else 0, QB):
                for n in range(FC // 2):
                    d_block(qc, n, D_ENGS[cp_i % len(D_ENGS)])
                    cp_i += 1

    legalize_waits(nc)
    return nc


_PROGRAMS = {}
BEST_KW = dict(ebufs=3, obufs=6)


def _get_program(with_bias: bool):
    key = with_bias
    if key not in _PROGRAMS:
        _PROGRAMS[key] = build_program(with_bias, **BEST_KW)
    return _PROGRAMS[key]


def _host_inputs(query, Wqkv, bqkv, Wo, bo, Xi):
    """Per-core input maps. Core c = (batch c//2, forecast-half c%2)."""
    query = np.asarray(query, np.float32)
    Wqkv = np.asarray(Wqkv, np.float32)
    bqkv = np.asarray(bqkv, np.float32)
    Wo = np.asarray(Wo, np.float32)
    bo = np.asarray(bo, np.float32)
    Xi = np.asarray(Xi, np.float64)

    # Wt[j, h] = (I + Xi_h - Xi_h^T)^(j+1) @ Wo_h, stacked over h.
    A = Xi - np.swapaxes(Xi, -1, -2)
    B = np.eye(EH, dtype=np.float64)[None] + A          # [H, 64, 64]
    Wt = np.empty((FC, E, E), np.float32)
    Bp = np.broadcast_to(np.eye(EH, dtype=np.float64), (H, EH, EH)).copy()
    Wo64 = Wo.astype(np.float64).reshape(H, EH, E)
    for j in range(FC):
        Bp = Bp @ B
        Wt[j] = (Bp @ Wo64).reshape(E, E).astype(np.float32)

    kk = np.arange(128)[:, None]
    qq = np.arange(128)[None, :]
    negm = np.where(qq >= kk, 0.0, -1e30).astype(ml_dtypes.bfloat16)
    iden = np.eye(128, dtype=ml_dtypes.bfloat16)

    onesr = np.ones((1, 512), np.float32)
    onesf = np.ones((1, 64), np.float32)
    bo_r = bo.reshape(1, -1)
    with_bias = bool(np.any(bqkv) or np.any(bo))

    in_maps = []
    for c in range(NCORES):
        b, g = c // 2, c % 2
        # this core owns heads 4g..4g+3: their q, k, v channel slices
        qs, ks, vs = (slice(g * EL, (g + 1) * EL),
                      slice(E + g * EL, E + (g + 1) * EL),
                      slice(2 * E + g * EL, 2 * E + (g + 1) * EL))
        wqk = np.ascontiguousarray(
            np.concatenate([Wqkv[:, qs], Wqkv[:, ks]], axis=1))
        wv_ = np.ascontiguousarray(Wqkv[:, vs])
        bqk_ = np.concatenate([bqkv[qs], bqkv[ks]]).reshape(1, -1)
        bv_ = np.ascontiguousarray(bqkv[vs]).reshape(1, -1)
        in_maps.append({
            "qT": np.ascontiguousarray(query[b].T).astype(ml_dtypes.bfloat16),
            "wqk": wqk.astype(ml_dtypes.bfloat16),
            "wv": wv_.astype(ml_dtypes.bfloat16),
            "wt": np.ascontiguousarray(Wt[4 * g: 4 * g + 4]).astype(
                ml_dtypes.bfloat16),
            "bqk": bqk_,
            "bv": bv_,
            "bo": bo_r,
            "onesr": onesr,
            "onesf": onesf,
            "negm": negm,
            "iden": iden,
        })
    return in_maps, with_bias


def _run(in_maps, with_bias, **kw):
    nc = _get_program(with_bias)
    return run_bass_kernel_spmd(nc, in_maps, list(range(NCORES)), **kw)


def kernel(query, key, value, Wqkv, bqkv, Wo, bo, Xi, _res_out=None, **kw):
    in_maps, with_bias = _host_inputs(query, Wqkv, bqkv, Wo, bo, Xi)
    res = _run(in_maps, with_bias, **kw)
    if _res_out is not None:
        _res_out.append(res)
    full = np.empty((N_B, FC, L, E), np.float32)
    for c in range(NCORES):
        b, g = c // 2, c % 2
        full[b, 4 * g: 4 * g + 4] = res.results[c]["out"].astype(np.float32)
    return full
